# revision 23
# baseline (speedup 1.0000x reference)
"""Trainium2 Bass kernel for nn_Net_78357383348450 (retrieval_knn).

Reference math:
    feats = x @ W                        # [ns, n_feat]
    preds = feats / max(||feats||, eps)  # row L2 norm (positive scale)
    out   = one_hot(argmin(-(preds @ means.T)), n_classes)

Since the per-row norm is a positive scale, argmin(-(preds @ means.T)) ==
argmax((x @ W) @ means.T); the normalization cannot change the winner.  The
device computes scores = (x @ W) @ means.T and emits one_hot(argmax(scores)).

One-hot trick: per 128-sample tile, DVE computes rowmax over the 1000 class
scores (still in PSUM), then ScalarE evaluates Exp(K*score - K*rowmax) with
K = 2**25 (a power of two, so K*rowmax is exact and ACT's fused scale-mult
cancels to exactly 0 for the winner -> Exp(0) = 1).  Losers see
K*(score-max) <= -167 (min abs score gap of this distribution ~5e-6) and
underflow to 0.

Precision modes per GEMM: "f32" uses the PE's native fp32 path (4 cyc/row).
"hilo" splits each operand into fp16 hi+lo (f = fh + fl exactly to ~2^-22
relative) and accumulates fh@mh + fh@ml + fl@mh in fp32 PSUM (3 cyc/row,
dropped fl@ml term ~2^-22 — far below the 1.4e-6 min top-2 argmax margin of
this distribution).

Sharding: data-parallel over samples, 2048 per core; W and means replicated.
x is transposed host-side so each core's GEMM1 needs no on-device transpose:
    featsT[m, s] = W[:, m].T @ xT[:, s]          (lhsT = W,      rhs = xT)
    scores[s, c] = featsT[:, s].T @ meansT[:, c] (lhsT = featsT, rhs = meansT)
"""

from contextlib import ExitStack

import numpy as np

import concourse.bass as bass
import concourse.mybir as mybir
import concourse.tile as tile
from concourse import bacc
from concourse.bass_utils import run_bass_kernel_spmd

N_CORES = 8
NS, D_IN, N_FEAT, N_CLASSES = 16384, 512, 256, 1000
SHARD = NS // N_CORES          # 2048 samples per core
BLK = 512                      # sample block (PSUM-friendly GEMM1 moving dim)
N_BLK = SHARD // BLK           # 4
TILES_PER_BLK = BLK // 128     # 4
K_SCALE = float(2 ** 25)       # power of two -> exact cancellation in ACT fma

# Tuning knobs (validated: f32/f32/f32 exact on HW; hilo exact in host sim).
GEMM1_MODE = "hilo"            # "f32" | "hilo"
GEMM2_MODE = "hilo"            # "f32" | "hilo" | "f32r"
OUT_DTYPE = "bf16"             # "f32" | "bf16" | "u8"
WARMUP_MM = 6                 # dummy matmuls during initial DMA wait (HAM warm)

F32 = mybir.dt.float32
F32R = mybir.dt.float32r
F16 = mybir.dt.float16
AX = mybir.AxisListType
AF = mybir.ActivationFunctionType

_OUT_DT = {"f32": F32, "bf16": mybir.dt.bfloat16, "u8": mybir.dt.uint8}
_OUT_NP = {"f32": np.float32, "bf16": None, "u8": np.uint8}


def _emit(tc, tensors):
    nc = tc.nc
    out = tensors["out"]
    KC = D_IN // 128           # 4 contraction chunks for GEMM1
    FC = N_FEAT // 128         # 2 contraction chunks for GEMM2
    oh_dt = _OUT_DT[OUT_DTYPE]

    with ExitStack() as ctx:
        const = ctx.enter_context(tc.tile_pool(name="const", bufs=1))
        xpool = ctx.enter_context(tc.tile_pool(name="x", bufs=2))
        ftps = ctx.enter_context(tc.tile_pool(name="ftps", bufs=2, space="PSUM"))
        ftsb = ctx.enter_context(tc.tile_pool(name="ftsb", bufs=2))
        scps = ctx.enter_context(tc.tile_pool(name="scps", bufs=2, space="PSUM"))
        ohsb = ctx.enter_context(tc.tile_pool(name="oh", bufs=3))
        small = ctx.enter_context(tc.tile_pool(name="small", bufs=4))

        def load_const(name, src, eng=None):
            # One DMA for all 128-row chunks: [(k p) f] -> SBUF [p, k, f].
            rows, cols = src.shape
            kc = rows // 128
            t = const.tile([128, kc, cols], src.dtype, tag=name)
            (eng or nc.sync).dma_start(
                t[:], src.rearrange("(k p) f -> p k f", p=128)
            )
            return [t[:, k, :] for k in range(kc)]

        def load_x(blk, name, src, dt, eng):
            # Per-block loads keep the first block's dependency small; the
            # two fp16 halves ride different HWDGE rings so dispatch
            # parallelizes at startup.  One DMA covers all KC chunks.
            t = xpool.tile([128, KC, BLK], dt, tag=name)
            eng.dma_start(
                t[:],
                src.rearrange("(k p) s -> p k s", p=128)[
                    :, :, blk * BLK:(blk + 1) * BLK
                ],
            )
            return [t[:, k, :] for k in range(KC)]

        def load_x_blk(blk):
            if GEMM1_MODE == "f32":
                return (load_x(blk, "x", tensors["xT"], F32, nc.sync),)
            return (
                load_x(blk, "xh", tensors["xTh"], F16, nc.sync),
                load_x(blk, "xl", tensors["xTl"], F16, nc.scalar),
            )

        # Dummy matmuls on a zeroed scratch tile while the first DMAs land:
        # keeps the PE busy from t=0 so the HAM clock gate is already at
        # 8/8 when real matmuls start (~3.4us of activity required).
        if WARMUP_MM:
            wsrc = const.tile([128, 512], F16, tag="warm_src")
            nc.gpsimd.memset(wsrc[:], 0.0)
            wps = ftps.tile([128, 512], F32, tag="warm_ps")
            for _ in range(WARMUP_MM):
                nc.tensor.matmul(
                    wps[:], wsrc[:, 0:128], wsrc[:], start=True, stop=True
                )

        # Replicated weights first (small, needed immediately), then block
        # 0's activations, then the prototype table (not needed until the
        # first GEMM2, ~8 matmuls in) — keeps the PE from waiting on DMA.
        if GEMM1_MODE == "f32":
            W_sb = load_const("W", tensors["W"])
        else:
            Wh_sb = load_const("Wh", tensors["Wh"])
            Wl_sb = load_const("Wl", tensors["Wl"], eng=nc.scalar)
        x_blk0 = load_x_blk(0)
        if GEMM2_MODE == "hilo":
            mh_sb = load_const("mh", tensors["meansTh"])
            ml_sb = load_const("ml", tensors["meansTl"], eng=nc.scalar)
        else:
            mT_sb = load_const("mT", tensors["meansT"])

        for blk in range(N_BLK):
            if blk == 0:
                x_tiles = x_blk0
            else:
                x_tiles = load_x_blk(blk)
            if GEMM1_MODE == "f32":
                (x_sb,) = x_tiles
            else:
                xh_sb, xl_sb = x_tiles
            bs = slice(0, BLK)

            # GEMM1: featsT chunks [128 feats, BLK samples], K = D_IN.
            ft_ps = []
            for m in range(FC):
                ps = ftps.tile([128, BLK], F32, tag="ftps")
                ms = slice(m * 128, (m + 1) * 128)
                if GEMM1_MODE == "f32":
                    for k in range(KC):
                        nc.tensor.matmul(
                            ps[:], W_sb[k][:, ms], x_sb[k][:, bs],
                            start=(k == 0), stop=(k == KC - 1),
                        )
                else:
                    passes = [(Wh_sb, xh_sb), (Wl_sb, xh_sb), (Wh_sb, xl_sb)]
                    for p, (Wp, xp) in enumerate(passes):
                        for k in range(KC):
                            nc.tensor.matmul(
                                ps[:], Wp[k][:, ms], xp[k][:, bs],
                                start=(p == 0 and k == 0),
                                stop=(p == 2 and k == KC - 1),
                            )
                ft_ps.append(ps)

            # featsT PSUM -> SBUF (fp32, or fp16 hi/lo split for GEMM2).
            if GEMM2_MODE == "hilo":
                fh_sb, fl_sb = [], []
                for m in range(FC):
                    fh = ftsb.tile([128, BLK], F16, tag=f"fh{m}")
                    nc.scalar.activation(fh[:], ft_ps[m][:], AF.Copy)
                    fl = ftsb.tile([128, BLK], F16, tag=f"fl{m}")
                    nc.vector.tensor_sub(fl[:], ft_ps[m][:], fh[:])
                    fh_sb.append(fh)
                    fl_sb.append(fl)
            else:
                ft_sb = []
                for m in range(FC):
                    t = ftsb.tile([128, BLK], F32, tag=f"ft{m}")
                    nc.scalar.activation(t[:], ft_ps[m][:], AF.Copy)
                    ft_sb.append(t)

            # GEMM2 + argmax + one-hot per 128-sample tile.
            for s in range(TILES_PER_BLK):
                ss = slice(s * 128, (s + 1) * 128)
                sc = scps.tile([128, 1024], F32, tag="scps")
                if GEMM2_MODE == "hilo":
                    passes = [
                        (fh_sb, mh_sb), (fh_sb, ml_sb), (fl_sb, mh_sb)
                    ]
                else:
                    passes = [(ft_sb, mT_sb)]
                np_ = len(passes)
                for p, (fp, mp) in enumerate(passes):
                    for k2 in range(FC):
                        st = (p == 0 and k2 == 0)
                        sp = (p == np_ - 1 and k2 == FC - 1)
                        lhsT = fp[k2][:, ss]
                        r0 = mp[k2][:, 0:512]
                        r1 = mp[k2][:, 512:N_CLASSES]
                        if GEMM2_MODE == "f32r":
                            lhsT = lhsT.bitcast(F32R)
                            r0 = r0.bitcast(F32R)
                            r1 = r1.bitcast(F32R)
                        nc.tensor.matmul(
                            sc[:, 0:512], lhsT, r0, start=st, stop=sp
                        )
                        nc.tensor.matmul(
                            sc[:, 512:N_CLASSES], lhsT, r1, start=st, stop=sp
                        )
                scores = sc[:, 0:N_CLASSES]
                rmax = small.tile([128, 1], F32, tag="rmax")
                nc.vector.reduce_max(rmax[:], scores, axis=AX.X)
                nbias = small.tile([128, 1], F32, tag="nbias")
                nc.gpsimd.tensor_scalar_mul(nbias[:], rmax[:], -K_SCALE)
                oh = ohsb.tile([128, N_CLASSES], oh_dt, tag="oh")
                nc.scalar.activation(
                    oh[:], scores, AF.Exp, bias=nbias[:], scale=K_SCALE
                )
                row = (blk * TILES_PER_BLK + s) * 128
                # Output DMAs ride the ScalarE HWDGE ring so they don't
                # queue behind the input loads on the sync ring.
                nc.scalar.dma_start(out[row:row + 128, :], oh[:])


_CACHE = {}


def _build():
    key = (GEMM1_MODE, GEMM2_MODE, OUT_DTYPE)
    if key in _CACHE:
        return _CACHE[key]
    nc = bacc.Bacc(
        "TRN2", target_bir_lowering=False, debug=False, num_devices=N_CORES
    )

    def din(name, shape, dt):
        return nc.dram_tensor(name, shape, dt, kind="ExternalInput").ap()

    tensors = {}
    if GEMM1_MODE == "f32":
        tensors["xT"] = din("xT", [D_IN, SHARD], F32)
        tensors["W"] = din("W", [D_IN, N_FEAT], F32)
    else:
        tensors["xTh"] = din("xTh", [D_IN, SHARD], F16)
        tensors["xTl"] = din("xTl", [D_IN, SHARD], F16)
        tensors["Wh"] = din("Wh", [D_IN, N_FEAT], F16)
        tensors["Wl"] = din("Wl", [D_IN, N_FEAT], F16)
    if GEMM2_MODE == "hilo":
        tensors["meansTh"] = din("meansTh", [N_FEAT, N_CLASSES], F16)
        tensors["meansTl"] = din("meansTl", [N_FEAT, N_CLASSES], F16)
    else:
        tensors["meansT"] = din("meansT", [N_FEAT, N_CLASSES], F32)
    tensors["out"] = nc.dram_tensor(
        "out", [SHARD, N_CLASSES], _OUT_DT[OUT_DTYPE], kind="ExternalOutput"
    ).ap()

    with tile.TileContext(nc) as tc:
        _emit(tc, tensors)
    nc.compile()
    _CACHE[key] = nc
    return nc


def _split16(a):
    hi = a.astype(np.float16)
    lo = (a - hi.astype(np.float32)).astype(np.float16)
    return hi, lo


def _in_maps(x, W, means):
    x = np.ascontiguousarray(x, dtype=np.float32)
    W = np.ascontiguousarray(W, dtype=np.float32)
    meansT = np.ascontiguousarray(np.asarray(means, dtype=np.float32).T)
    shared = {}
    if GEMM1_MODE == "f32":
        shared["W"] = W
    else:
        shared["Wh"], shared["Wl"] = _split16(W)
    if GEMM2_MODE == "hilo":
        shared["meansTh"], shared["meansTl"] = _split16(meansT)
    else:
        shared["meansT"] = meansT
    maps = []
    for i in range(N_CORES):
        xTi = np.ascontiguousarray(x[i * SHARD:(i + 1) * SHARD, :].T)
        m = dict(shared)
        if GEMM1_MODE == "f32":
            m["xT"] = xTi
        else:
            m["xTh"], m["xTl"] = _split16(xTi)
        maps.append(m)
    return maps


def run(x, W, means, trace=False, **spmd_kwargs):
    nc = _build()
    res = run_bass_kernel_spmd(
        nc, _in_maps(x, W, means), list(range(N_CORES)),
        trace=trace, **spmd_kwargs,
    )
    full = np.concatenate(
        [res.results[i]["out"] for i in range(N_CORES)], axis=0
    )
    return np.ascontiguousarray(full.astype(np.float32)), res


def kernel(x, W, means, t=None, **_unused):
    out, _ = run(x, W, means, trace=False)
    return out


# revision 28
# speedup vs baseline: 1.1675x; 1.1675x over previous
"""Trainium2 Bass kernel for nn_Net_78357383348450 (retrieval_knn).

Reference math:
    feats = x @ W                        # [ns, n_feat]
    preds = feats / max(||feats||, eps)  # row L2 norm (positive scale)
    out   = one_hot(argmin(-(preds @ means.T)), n_classes)

Since the per-row norm is a positive scale, argmin(-(preds @ means.T)) ==
argmax((x @ W) @ means.T); the normalization cannot change the winner.  The
device computes scores = (x @ W) @ means.T and emits one_hot(argmax(scores)).

One-hot trick: per 128-sample tile, DVE computes rowmax over the 1000 class
scores (still in PSUM), then ScalarE evaluates Exp(K*score - K*rowmax) with
K = 2**25 (a power of two, so K*rowmax is exact and ACT's fused scale-mult
cancels to exactly 0 for the winner -> Exp(0) = 1).  Losers see
K*(score-max) <= -167 (min abs score gap of this distribution ~5e-6) and
underflow to 0.

Precision modes per GEMM: "f32" uses the PE's native fp32 path (4 cyc/row).
"hilo" splits each operand into fp16 hi+lo (f = fh + fl exactly to ~2^-22
relative) and accumulates fh@mh + fh@ml + fl@mh in fp32 PSUM (3 cyc/row,
dropped fl@ml term ~2^-22 — far below the 1.4e-6 min top-2 argmax margin of
this distribution).

Sharding: data-parallel over samples, 2048 per core; W and means replicated.
x is transposed host-side so each core's GEMM1 needs no on-device transpose:
    featsT[m, s] = W[:, m].T @ xT[:, s]          (lhsT = W,      rhs = xT)
    scores[s, c] = featsT[:, s].T @ meansT[:, c] (lhsT = featsT, rhs = meansT)
"""

from contextlib import ExitStack

import numpy as np

import concourse.bass as bass
import concourse.mybir as mybir
import concourse.tile as tile
from concourse import bacc
from concourse.bass_utils import run_bass_kernel_spmd

N_CORES = 8
NS, D_IN, N_FEAT, N_CLASSES = 16384, 512, 256, 1000
SHARD = NS // N_CORES          # 2048 samples per core
# Sample block sizes (PSUM-friendly GEMM1 moving dims, each <= 512).  Two
# small lead blocks let the PE start while the rest of x streams in.
BLKS = (256, 256, 512, 512, 512)
assert sum(BLKS) == SHARD
K_SCALE = float(2 ** 25)       # power of two -> exact cancellation in ACT fma

# Tuning knobs (validated: f32/f32/f32 exact on HW; hilo exact in host sim).
GEMM1_MODE = "hilo"            # "f32" | "hilo"
GEMM2_MODE = "hilo"            # "f32" | "hilo" | "f32r"
OUT_DTYPE = "bf16"             # "f32" | "bf16" | "u8"
WARMUP_MM = 6                 # dummy matmuls during initial DMA wait (HAM warm)

F32 = mybir.dt.float32
F32R = mybir.dt.float32r
F16 = mybir.dt.float16
AX = mybir.AxisListType
AF = mybir.ActivationFunctionType

_OUT_DT = {"f32": F32, "bf16": mybir.dt.bfloat16, "u8": mybir.dt.uint8}
_OUT_NP = {"f32": np.float32, "bf16": None, "u8": np.uint8}


def _emit(tc, tensors):
    nc = tc.nc
    out = tensors["out"]
    KC = D_IN // 128           # 4 contraction chunks for GEMM1
    FC = N_FEAT // 128         # 2 contraction chunks for GEMM2
    oh_dt = _OUT_DT[OUT_DTYPE]

    with ExitStack() as ctx:
        const = ctx.enter_context(tc.tile_pool(name="const", bufs=1))
        xpool = ctx.enter_context(tc.tile_pool(name="x", bufs=2))
        ftps = ctx.enter_context(tc.tile_pool(name="ftps", bufs=2, space="PSUM"))
        ftsb = ctx.enter_context(tc.tile_pool(name="ftsb", bufs=2))
        scps = ctx.enter_context(tc.tile_pool(name="scps", bufs=2, space="PSUM"))
        ohsb = ctx.enter_context(tc.tile_pool(name="oh", bufs=3))
        small = ctx.enter_context(tc.tile_pool(name="small", bufs=4))

        def load_const(name, src, eng=None):
            # One DMA for all 128-row chunks: [(k p) f] -> SBUF [p, k, f].
            rows, cols = src.shape
            kc = rows // 128
            t = const.tile([128, kc, cols], src.dtype, tag=name)
            (eng or nc.sync).dma_start(
                t[:], src.rearrange("(k p) f -> p k f", p=128)
            )
            return [t[:, k, :] for k in range(kc)]

        def load_x(s0, blk, name, src, dt, eng):
            # Per-block loads keep the first block's dependency small; the
            # two fp16 halves ride different HWDGE rings so dispatch
            # parallelizes at startup.  One DMA covers all KC chunks.
            t = xpool.tile([128, KC, blk], dt, tag=name)
            eng.dma_start(
                t[:],
                src.rearrange("(k p) s -> p k s", p=128)[:, :, s0:s0 + blk],
            )
            return [t[:, k, :] for k in range(KC)]

        def load_x_blk(s0, blk):
            if GEMM1_MODE == "f32":
                return (load_x(s0, blk, "x", tensors["xT"], F32, nc.sync),)
            return (
                load_x(s0, blk, "xh", tensors["xTh"], F16, nc.sync),
                load_x(s0, blk, "xl", tensors["xTl"], F16, nc.scalar),
            )

        # Dummy matmuls on a zeroed scratch tile while the first DMAs land:
        # keeps the PE busy from t=0 so the HAM clock gate is already at
        # 8/8 when real matmuls start (~3.4us of activity required).
        if WARMUP_MM:
            wsrc = const.tile([128, 512], F16, tag="warm_src")
            nc.gpsimd.memset(wsrc[:], 0.0)
            wps = ftps.tile([128, 512], F32, tag="warm_ps")
            for _ in range(WARMUP_MM):
                nc.tensor.matmul(
                    wps[:], wsrc[:, 0:128], wsrc[:], start=True, stop=True
                )

        # Replicated weights first (small, needed immediately), then block
        # 0's activations, then the prototype table (not needed until the
        # first GEMM2, ~8 matmuls in) — keeps the PE from waiting on DMA.
        if GEMM1_MODE == "f32":
            W_sb = load_const("W", tensors["W"])
        else:
            Wh_sb = load_const("Wh", tensors["Wh"])
            Wl_sb = load_const("Wl", tensors["Wl"], eng=nc.scalar)
        x_blk0 = load_x_blk(0, BLKS[0])
        if GEMM2_MODE == "hilo":
            mh_sb = load_const("mh", tensors["meansTh"])
            ml_sb = load_const("ml", tensors["meansTl"], eng=nc.scalar)
        else:
            mT_sb = load_const("mT", tensors["meansT"])

        s_base = 0
        for bi, blk in enumerate(BLKS):
            if bi == 0:
                x_tiles = x_blk0
            else:
                x_tiles = load_x_blk(s_base, blk)
            if GEMM1_MODE == "f32":
                (x_sb,) = x_tiles
            else:
                xh_sb, xl_sb = x_tiles
            bs = slice(0, blk)

            # GEMM1: featsT chunks [128 feats, blk samples], K = D_IN.
            ft_ps = []
            for m in range(FC):
                ps = ftps.tile([128, blk], F32, tag="ftps")
                ms = slice(m * 128, (m + 1) * 128)
                if GEMM1_MODE == "f32":
                    for k in range(KC):
                        nc.tensor.matmul(
                            ps[:], W_sb[k][:, ms], x_sb[k][:, bs],
                            start=(k == 0), stop=(k == KC - 1),
                        )
                else:
                    passes = [(Wh_sb, xh_sb), (Wl_sb, xh_sb), (Wh_sb, xl_sb)]
                    for p, (Wp, xp) in enumerate(passes):
                        for k in range(KC):
                            nc.tensor.matmul(
                                ps[:], Wp[k][:, ms], xp[k][:, bs],
                                start=(p == 0 and k == 0),
                                stop=(p == 2 and k == KC - 1),
                            )
                ft_ps.append(ps)

            # featsT PSUM -> SBUF (fp32, or fp16 hi/lo split for GEMM2).
            if GEMM2_MODE == "hilo":
                fh_sb, fl_sb = [], []
                for m in range(FC):
                    fh = ftsb.tile([128, blk], F16, tag=f"fh{m}")
                    nc.scalar.activation(fh[:], ft_ps[m][:], AF.Copy)
                    fl = ftsb.tile([128, blk], F16, tag=f"fl{m}")
                    nc.vector.tensor_sub(fl[:], ft_ps[m][:], fh[:])
                    fh_sb.append(fh)
                    fl_sb.append(fl)
            else:
                ft_sb = []
                for m in range(FC):
                    t = ftsb.tile([128, blk], F32, tag=f"ft{m}")
                    nc.scalar.activation(t[:], ft_ps[m][:], AF.Copy)
                    ft_sb.append(t)

            # GEMM2 + argmax + one-hot per 128-sample tile.
            for s in range(blk // 128):
                ss = slice(s * 128, (s + 1) * 128)
                sc = scps.tile([128, 1024], F32, tag="scps")
                if GEMM2_MODE == "hilo":
                    passes = [
                        (fh_sb, mh_sb), (fh_sb, ml_sb), (fl_sb, mh_sb)
                    ]
                else:
                    passes = [(ft_sb, mT_sb)]
                np_ = len(passes)
                for p, (fp, mp) in enumerate(passes):
                    for k2 in range(FC):
                        st = (p == 0 and k2 == 0)
                        sp = (p == np_ - 1 and k2 == FC - 1)
                        lhsT = fp[k2][:, ss]
                        r0 = mp[k2][:, 0:512]
                        r1 = mp[k2][:, 512:N_CLASSES]
                        if GEMM2_MODE == "f32r":
                            lhsT = lhsT.bitcast(F32R)
                            r0 = r0.bitcast(F32R)
                            r1 = r1.bitcast(F32R)
                        nc.tensor.matmul(
                            sc[:, 0:512], lhsT, r0, start=st, stop=sp
                        )
                        nc.tensor.matmul(
                            sc[:, 512:N_CLASSES], lhsT, r1, start=st, stop=sp
                        )
                scores = sc[:, 0:N_CLASSES]
                rmax = small.tile([128, 1], F32, tag="rmax")
                nc.vector.reduce_max(rmax[:], scores, axis=AX.X)
                nbias = small.tile([128, 1], F32, tag="nbias")
                nc.gpsimd.tensor_scalar_mul(nbias[:], rmax[:], -K_SCALE)
                oh = ohsb.tile([128, N_CLASSES], oh_dt, tag="oh")
                nc.scalar.activation(
                    oh[:], scores, AF.Exp, bias=nbias[:], scale=K_SCALE
                )
                row = s_base + s * 128
                # Output DMAs ride the ScalarE HWDGE ring so they don't
                # queue behind the input loads on the sync ring.
                nc.scalar.dma_start(out[row:row + 128, :], oh[:])
            s_base += blk


_CACHE = {}


def _build():
    key = (GEMM1_MODE, GEMM2_MODE, OUT_DTYPE)
    if key in _CACHE:
        return _CACHE[key]
    nc = bacc.Bacc(
        "TRN2", target_bir_lowering=False, debug=False, num_devices=N_CORES
    )

    def din(name, shape, dt):
        return nc.dram_tensor(name, shape, dt, kind="ExternalInput").ap()

    tensors = {}
    if GEMM1_MODE == "f32":
        tensors["xT"] = din("xT", [D_IN, SHARD], F32)
        tensors["W"] = din("W", [D_IN, N_FEAT], F32)
    else:
        tensors["xTh"] = din("xTh", [D_IN, SHARD], F16)
        tensors["xTl"] = din("xTl", [D_IN, SHARD], F16)
        tensors["Wh"] = din("Wh", [D_IN, N_FEAT], F16)
        tensors["Wl"] = din("Wl", [D_IN, N_FEAT], F16)
    if GEMM2_MODE == "hilo":
        tensors["meansTh"] = din("meansTh", [N_FEAT, N_CLASSES], F16)
        tensors["meansTl"] = din("meansTl", [N_FEAT, N_CLASSES], F16)
    else:
        tensors["meansT"] = din("meansT", [N_FEAT, N_CLASSES], F32)
    tensors["out"] = nc.dram_tensor(
        "out", [SHARD, N_CLASSES], _OUT_DT[OUT_DTYPE], kind="ExternalOutput"
    ).ap()

    with tile.TileContext(nc) as tc:
        _emit(tc, tensors)
    nc.compile()
    _CACHE[key] = nc
    return nc


def _split16(a):
    hi = a.astype(np.float16)
    lo = (a - hi.astype(np.float32)).astype(np.float16)
    return hi, lo


def _in_maps(x, W, means):
    x = np.ascontiguousarray(x, dtype=np.float32)
    W = np.ascontiguousarray(W, dtype=np.float32)
    meansT = np.ascontiguousarray(np.asarray(means, dtype=np.float32).T)
    shared = {}
    if GEMM1_MODE == "f32":
        shared["W"] = W
    else:
        shared["Wh"], shared["Wl"] = _split16(W)
    if GEMM2_MODE == "hilo":
        shared["meansTh"], shared["meansTl"] = _split16(meansT)
    else:
        shared["meansT"] = meansT
    maps = []
    for i in range(N_CORES):
        xTi = np.ascontiguousarray(x[i * SHARD:(i + 1) * SHARD, :].T)
        m = dict(shared)
        if GEMM1_MODE == "f32":
            m["xT"] = xTi
        else:
            m["xTh"], m["xTl"] = _split16(xTi)
        maps.append(m)
    return maps


def run(x, W, means, trace=False, **spmd_kwargs):
    nc = _build()
    res = run_bass_kernel_spmd(
        nc, _in_maps(x, W, means), list(range(N_CORES)),
        trace=trace, **spmd_kwargs,
    )
    full = np.concatenate(
        [res.results[i]["out"] for i in range(N_CORES)], axis=0
    )
    return np.ascontiguousarray(full.astype(np.float32)), res


def kernel(x, W, means, t=None, **_unused):
    out, _ = run(x, W, means, trace=False)
    return out
